# revision 33
# baseline (speedup 1.0000x reference)
"""Multi-head causal attention (B=2, S=2048, D=1024, H=16) on 8 TRN2 NeuronCores.

Sharding: core c handles batch b = c//4 and head-group g = c%4 (4 heads, 256 dims).
Each core computes Q/K/V projections for its head group from x[b], runs causal
attention per head, and applies its 256 rows of Wo, producing a partial [S, D]
output (bf16). The host sums the 4 head-group partials per batch in fp32.

Device schedule (per core); matmul operands bf16, accumulation fp32 in PSUM.
One sweep over the four 512-wide i-chunks, the two head-pairs interleaved per
chunk so the ScalarE exp stream and the TensorE matmul stream both stay dense:

  for c in 0..3:  for pair in 0,1:
    scores S^T[j,i] per 128-row j-block, both heads issued back-to-back on
    disjoint PE row groups (K=64 pairing); 1-group-ahead software pipelining
    (scores for block g+1 are emitted before PV of block g so the PE never
    heads-of-line-waits on the exp)
    P~^T = exp(scale*S^T) (ScalarE, 2 strips per instruction)
    diagonal blocks causal-masked in-place on GpSimd (affine_select -> 0)
    O'^T[65,i] += V'_j^T @ P~^T_j  (PSUM accumulate; row 64 = softmax denom)
    normalize: copy nums+den to SBUF, reciprocal_approx_fast on the denom row,
    DRAM round-trip broadcasts the recip across the 64 head-dim partitions,
    oT = num * recip (head 1 of the pair lands via a partition-shift DMA)
  after both pairs of chunk c: y rows = O @ Wo (PSUM accumulate over pairs),
  copied out as bf16 and DMA'd per 128-row block.

Q/K/V and output projections are emitted as ordered "filler" work drained
between attention blocks to fill the PE's slack under the exp stream.
A warmup burst of dummy matmuls plus an early dummy activation run during the
input DMA phase so the PE's HAM clock-gate is released (2.4 GHz) and the exp
table is resident before real work arrives.
"""

import os
from collections import deque

import ml_dtypes
import numpy as np

import concourse.bass as bass
import concourse.mybir as mybir
import concourse.tile as tile
from concourse.bass_utils import run_bass_kernel_spmd

F32 = mybir.dt.float32
BF16 = mybir.dt.bfloat16

B, S, D, H = 2, 2048, 1024, 16
HD = 64                     # head dim
GH = 4                      # heads per core
GC = GH * HD                # 256 projection cols per core
P = 128
KD = D // P                 # 8 contraction chunks for projections
NSB = S // P                # 16 seq blocks
CHW = 512                   # i-chunk width
NCH = S // CHW              # 4 i-chunks
SCALE = HD ** -0.5

_NC_CACHE = None
LAST_RESULTS = None         # BassKernelResults of the most recent run (for test.py)


class _Fillers:
    """Queue of small emission closures (1-2 TensorE ops each) drained
    between attention strip groups to keep the PE busy while ScalarE
    works through the exp stream. Markers let the consumer force-drain
    the prefix a dependent phase needs."""

    def __init__(self):
        self.q = deque()       # static projection work, with markers
        self.hq = deque()      # dynamic work (out-projection), served first
        self.seen = set()      # markers already popped (by step or drains)
        self.keepwarm = None   # fallback emitter for empty queues
        self.kw_budget = 0

    def add(self, fn):
        self.q.append(fn)

    def add_hq(self, fn):
        self.hq.append(fn)

    def add_marker(self, key):
        self.q.append(key)

    def _emit_q_one(self):
        item = self.q.popleft()
        if callable(item):
            item()
            return None
        self.seen.add(item)
        return item

    def step(self, n):
        done = 0
        use_hq = True
        while done < n:
            if not (self.q or self.hq):
                # keep the PE's HAM clock-gate from re-throttling during
                # ScalarE-bound stretches: emit capped junk matmuls
                if self.keepwarm is not None and self.kw_budget > 0:
                    self.kw_budget -= 1
                    self.keepwarm()
                    done += 1
                    continue
                break
            if self.hq and (use_hq or not self.q):
                self.hq.popleft()()
                done += 1
            elif self.q:
                if self._emit_q_one() is None:
                    done += 1
            use_hq = not use_hq

    def drain_until(self, key):
        if key in self.seen:
            return
        while self.q:
            if self._emit_q_one() == key:
                return

    def drain(self):
        while self.q or self.hq:
            if self.hq:
                self.hq.popleft()()
            if self.q:
                self._emit_q_one()


def _emit(tc):
    nc = tc.nc
    xT = nc.dram_tensor("xT", [D, S], BF16, kind="ExternalInput")
    wq = nc.dram_tensor("wq", [D, GC], BF16, kind="ExternalInput")
    wk = nc.dram_tensor("wk", [D, GC], BF16, kind="ExternalInput")
    wv = nc.dram_tensor("wv", [D, GC], BF16, kind="ExternalInput")
    wo = nc.dram_tensor("wo", [GC, D], BF16, kind="ExternalInput")
    y = nc.dram_tensor("y", [S, D], BF16, kind="ExternalOutput")

    xT_t = xT[:].rearrange("(o p) s -> p o s", p=P)      # [128, 8, S]
    wq_t = wq[:].rearrange("(o p) c -> p o c", p=P)      # [128, 8, 256]
    wk_t = wk[:].rearrange("(o p) c -> p o c", p=P)
    wv_t = wv[:].rearrange("(o p) c -> p o c", p=P)
    wo_t = wo[:].rearrange("(o p) n -> p o n", p=P)      # [128, 2, 1024]

    from contextlib import ExitStack

    with ExitStack() as top:
        persist = top.enter_context(tc.tile_pool(name="persist", bufs=1))

        wdum = persist.tile([P, P], BF16)
        nc.vector.memset(wdum, 0.0)
        ones_bf = persist.tile([P, 1], BF16)
        nc.vector.memset(ones_bf, 1.0)
        ones64 = persist.tile([P, HD], BF16)      # K=1 broadcast lhsT rows
        nc.vector.memset(ones64, 1.0)
        ident = persist.tile([HD, HD], BF16)      # partition-shift identity
        nc.gpsimd.memset(ident, 1.0)
        nc.gpsimd.affine_select(
            out=ident, in_=ident, compare_op=mybir.AluOpType.is_equal,
            fill=0.0, base=0, pattern=[[-1, HD]], channel_multiplier=1)

        wq_sb = persist.tile([P, KD, GC], BF16)
        wk_sb = persist.tile([P, KD, GC], BF16)
        wv_sb = persist.tile([P, KD, GC], BF16)
        wo_sb = persist.tile([P, 2, D], BF16)
        xfull = persist.tile([P, KD, S], BF16)

        # input DMAs split across the sync/scalar HWDGE queues and the
        # gpsimd SWDGE, in strict first-needed order: Q/K weights and x
        # chunk 0 (so the first projections can start ~12us in), then wv,
        # then the later x chunks. Scalar only carries early issues so
        # the exp stream is undisturbed once attention starts.
        nc.sync.dma_start(out=wq_sb, in_=wq_t)
        nc.scalar.dma_start(out=wk_sb, in_=wk_t)
        for k in range(KD):
            eng = nc.sync if k % 2 == 0 else nc.scalar
            eng.dma_start(out=xfull[:, k, 0:CHW], in_=xT_t[:, k, 0:CHW])
        nc.sync.dma_start(out=wv_sb, in_=wv_t)
        for ch in range(1, NCH):
            nc.sync.dma_start(
                out=xfull[:, 0:4, ch * CHW:(ch + 1) * CHW],
                in_=xT_t[:, 0:4, ch * CHW:(ch + 1) * CHW])
            nc.scalar.dma_start(
                out=xfull[:, 4:8, ch * CHW:(ch + 1) * CHW],
                in_=xT_t[:, 4:8, ch * CHW:(ch + 1) * CHW])
        nc.scalar.dma_start(out=wo_sb, in_=wo_t)

        qT = persist.tile([P, 2, S], BF16)               # [pair-cols, pair, seq]
        kT = persist.tile([P, 2, S], BF16)
        v_sb = persist.tile([P, NSB, GH, HD + 1], BF16)  # ones col appended
        oT = persist.tile([P, 2, S], BF16)
        nc.vector.tensor_copy(
            out=v_sb[:, :, :, HD:HD + 1],
            in_=ones_bf[:, 0:1].to_broadcast((P, NSB, GH, 1)))

        with ExitStack() as ph_b:
            ps_sc = ph_b.enter_context(
                tc.tile_pool(name="ps_sc", bufs=2, space="PSUM"))
            ps_pv = ph_b.enter_context(
                tc.tile_pool(name="ps_pv", bufs=1, space="PSUM"))
            ps_fill = ph_b.enter_context(
                tc.tile_pool(name="ps_fill", bufs=2, space="PSUM"))
            ppool = ph_b.enter_context(tc.tile_pool(name="pstrip", bufs=3))
            npool = ph_b.enter_context(tc.tile_pool(name="norm", bufs=6))
            opool = ph_b.enter_context(tc.tile_pool(name="onum", bufs=4))
            ypool = ph_b.enter_context(tc.tile_pool(name="ystage", bufs=6))
            ypool32 = ph_b.enter_context(tc.tile_pool(name="ystage32", bufs=4))

            # --- warmup: release the PE clock gate and preload the exp
            # table while the input DMAs are in flight ---
            warm_act = persist.tile([P, 8], F32)
            nc.scalar.activation(
                warm_act, wdum[:, 0:8], mybir.ActivationFunctionType.Exp)
            wt = ps_fill.tile([P, P], F32, tag="fill", name="warm")
            NWARM = 90
            for i in range(NWARM):
                nc.tensor.matmul(wt, wdum, wdum,
                                 start=(i == 0), stop=(i == NWARM - 1))

            f = _Fillers()

            def _keepwarm_item():
                kw = ps_fill.tile([P, P], F32, tag="fill", name="kw")
                nc.tensor.matmul(kw, wdum, wdum)

            f.keepwarm = _keepwarm_item
            f.kw_budget = 0

            def _qk_chunk(which, pair_, ch):
                # which: 0=Q, 1=K; emits 8 accumulating matmuls + copy-out
                cell = {}
                w_sb = wq_sb if which == 0 else wk_sb
                dst = qT if which == 0 else kT

                def alloc_mm(k, cell=cell, ch=ch, w_sb=w_sb, pair_=pair_):
                    if k == 0:
                        cell["p"] = ps_fill.tile(
                            [P, CHW], F32, tag="fill", name="fillqk")
                    nc.tensor.matmul(
                        cell["p"], w_sb[:, k, pair_ * P:(pair_ + 1) * P],
                        xfull[:, k, ch * CHW:(ch + 1) * CHW],
                        start=(k == 0), stop=(k == KD - 1))

                def copy(cell=cell, ch=ch, dst=dst, pair_=pair_):
                    nc.vector.tensor_copy(
                        out=dst[:, pair_, ch * CHW:(ch + 1) * CHW],
                        in_=cell["p"])

                for k in range(KD):
                    f.add(lambda k=k: alloc_mm(k))
                f.add(copy)

            def _v_block(sb):
                cell = {}

                def alloc_mm(k, cell=cell, sb=sb):
                    if k == 0:
                        cell["pv"] = ps_fill.tile(
                            [P, CHW], F32, tag="fill", name="fillpv")
                    nc.tensor.matmul(
                        cell["pv"][:, 0:GC],
                        xfull[:, k, sb * P:(sb + 1) * P], wv_sb[:, k, :],
                        start=(k == 0), stop=(k == KD - 1))

                def copy(cell=cell, sb=sb):
                    nc.vector.tensor_copy(
                        out=v_sb[:, sb, :, 0:HD],
                        in_=cell["pv"][:, 0:GC].rearrange(
                            "p (h d) -> p h d", h=GH))

                for k in range(KD):
                    f.add(lambda k=k: alloc_mm(k))
                f.add(copy)

            def _outproj_gc0(c):
                # first half of the last chunk's output projection: the
                # pair-0 contraction runs during (3,1) attention into f32
                # staging, so the tail only runs the pair-1 matmuls + adds
                cells = {}
                for s4 in range(CHW // P):
                    sb = c * (CHW // P) + s4
                    cell = {}
                    cells[sb] = cell

                    def ph1(cell=cell, sb=sb):
                        cell["y32"] = ypool32.tile(
                            [P, D], F32, tag="y32", name="y32")

                    f.add_hq(ph1)
                    for nch in range(2):
                        def mm0(cell=cell, sb=sb, nch=nch):
                            cell["py"] = ps_fill.tile(
                                [P, CHW], F32, tag="fill", name="fillpy")
                            nc.tensor.matmul(
                                cell["py"], oT[:, 0, sb * P:(sb + 1) * P],
                                wo_sb[:, 0, nch * CHW:(nch + 1) * CHW])

                        def cp0(cell=cell, nch=nch):
                            nc.vector.tensor_copy(
                                out=cell["y32"][:, nch * CHW:(nch + 1) * CHW],
                                in_=cell["py"])

                        f.add_hq(mm0)
                        f.add_hq(cp0)
                return cells

            def _outproj_gc1(c, cells):
                for s4 in range(CHW // P):
                    sb = c * (CHW // P) + s4
                    cell = cells[sb]

                    def alloc(cell=cell):
                        cell["ysb"] = ypool.tile(
                            [P, D], BF16, tag="ysb", name="ysb")

                    f.add_hq(alloc)
                    for nch in range(2):
                        def mm1(cell=cell, sb=sb, nch=nch):
                            cell["py2"] = ps_fill.tile(
                                [P, CHW], F32, tag="fill", name="fillpy2")
                            nc.tensor.matmul(
                                cell["py2"], oT[:, 1, sb * P:(sb + 1) * P],
                                wo_sb[:, 1, nch * CHW:(nch + 1) * CHW])

                        def addcp(cell=cell, nch=nch):
                            nc.vector.tensor_add(
                                cell["ysb"][:, nch * CHW:(nch + 1) * CHW],
                                cell["y32"][:, nch * CHW:(nch + 1) * CHW],
                                cell["py2"])

                        f.add_hq(mm1)
                        f.add_hq(addcp)

                    def out_dma(cell=cell, sb=sb):
                        nc.sync.dma_start(
                            out=y[sb * P:(sb + 1) * P, :], in_=cell["ysb"])

                    f.add_hq(out_dma)

            def _outproj_chunk(c):
                for s4 in range(CHW // P):
                    sb = c * (CHW // P) + s4
                    cell = {}

                    def alloc(cell=cell):
                        cell["ysb"] = ypool.tile(
                            [P, D], BF16, tag="ysb", name="ysb")

                    f.add_hq(alloc)
                    for nch in range(2):
                        def mm(gc, cell=cell, sb=sb, nch=nch):
                            if gc == 0:
                                cell["py"] = ps_fill.tile(
                                    [P, CHW], F32, tag="fill", name="fillpy")
                            nc.tensor.matmul(
                                cell["py"], oT[:, gc, sb * P:(sb + 1) * P],
                                wo_sb[:, gc, nch * CHW:(nch + 1) * CHW],
                                start=(gc == 0), stop=(gc == 1))

                        def cp(cell=cell, nch=nch):
                            nc.vector.tensor_copy(
                                out=cell["ysb"][:, nch * CHW:(nch + 1) * CHW],
                                in_=cell["py"])

                        f.add_hq(lambda mm=mm: mm(0))
                        f.add_hq(lambda mm=mm: mm(1))
                        f.add_hq(cp)

                    def out_dma(cell=cell, sb=sb):
                        nc.sync.dma_start(
                            out=y[sb * P:(sb + 1) * P, :], in_=cell["ysb"])

                    f.add_hq(out_dma)

            # projection work, in first-needed order. Markers are placed so
            # a chunk only force-drains Q at its start; K and the V blocks
            # are only forced right before the diagonal groups that first
            # need them, letting them spread across the chunk's groups.
            for ch in range(NCH):
                _qk_chunk(0, 0, ch)
                f.add_marker(("q", 0, ch))
                _qk_chunk(0, 1, ch)
                f.add_marker(("q", 1, ch))
                _qk_chunk(1, 0, ch)
                f.add_marker(("k", 0, ch))
                _qk_chunk(1, 1, ch)
                f.add_marker(("k", 1, ch))
                for s4 in range(CHW // P):
                    _v_block(ch * (CHW // P) + s4)
                f.add_marker(("v", ch))

            def _attn_chunk(c, pair):
                njb = 4 * c + 4
                pvacc = {
                    0: ps_pv.tile([P, CHW], F32, tag="pv0", name="pv0"),
                    1: ps_pv.tile([P, CHW], F32, tag="pv1", name="pv1"),
                }
                scs = {}

                def emit_scores(jb):
                    if jb == 4 * c:     # diagonal: K(pair,c) now needed
                        f.drain_until(("k", pair, c))
                    tl = max(0, jb - 4 * c) * P
                    sc = ps_sc.tile([P, 2, CHW], F32, tag="sc", name="sc")
                    for hp in (0, 1):
                        bp = hp * HD
                        nc.tensor.matmul(
                            sc[:, hp, tl:],
                            kT[bp:bp + HD, pair, jb * P:(jb + 1) * P],
                            qT[bp:bp + HD, pair, c * CHW + tl:(c + 1) * CHW])
                    scs[jb] = (sc, tl)

                emit_scores(0)
                for jb in range(njb):
                    sc, tl = scs.pop(jb)
                    pt = ppool.tile([P, 2, CHW], BF16, tag="pt", name="pt")
                    nc.scalar.activation(
                        pt[:, :, tl:], sc[:, :, tl:],
                        mybir.ActivationFunctionType.Exp, scale=SCALE)
                    if jb + 1 < njb:
                        emit_scores(jb + 1)
                    if jb >= 4 * c:           # diagonal block: causal mask
                        nc.gpsimd.affine_select(
                            out=pt[:, :, tl:tl + P], in_=pt[:, :, tl:tl + P],
                            compare_op=mybir.AluOpType.is_ge, fill=0.0,
                            base=0, pattern=[[0, 2], [1, P]],
                            channel_multiplier=-1)
                    if pair == 0 and jb == 4 * c:   # diag PV needs V(c)
                        f.drain_until(("v", c))
                    f.step(6)
                    for hp in (0, 1):
                        h = pair * 2 + hp
                        nc.tensor.matmul(
                            pvacc[hp][0:HD + 1, tl:], v_sb[:, jb, h, :],
                            pt[:, hp, tl:],
                            start=(jb == 0), stop=(jb == njb - 1))
                return pvacc

            def _normalize(c, pair, pvacc):
                # DMA-free normalize (every chunk): copy nums+den to SBUF,
                # reciprocal of the two denominator rows as exp(-ln(den))
                # on ScalarE (one batched instruction each; Ln and Exp
                # share the natural_log_exp table set), partition-broadcast
                # of the recip via a K=1 TensorE matmul into the freed
                # pvacc bank, and the head-1 partition shift via an
                # identity matmul. The ~10us-per-hop DMA latency of the
                # runtime never enters the oT dependency chain.
                ccols = slice(c * CHW, (c + 1) * CHW)
                onum = opool.tile([HD + 1, 2, CHW], F32, tag="on", name="onum")
                for hp in (0, 1):
                    nc.scalar.activation(
                        onum[HD:HD + 1, hp, :], pvacc[hp][HD:HD + 1, :],
                        mybir.ActivationFunctionType.Ln)
                    nc.vector.tensor_copy(
                        out=onum[0:HD, hp, :], in_=pvacc[hp][0:HD, :])
                den = onum[HD:HD + 1, :, :]
                rcpb = opool.tile([HD + 1, 2, CHW], BF16, tag="rcb",
                                  name="rcpb")
                nc.scalar.activation(
                    rcpb[HD:HD + 1, :, :], den,
                    mybir.ActivationFunctionType.Exp, scale=-1.0)
                for hp in (0, 1):
                    # broadcast recip across the 64 head-dim rows into the
                    # freed pvacc bank (start=True only clears has_written
                    # bits; no live data remains there); bf16 operands keep
                    # the matmul single-pass
                    nc.tensor.matmul(
                        pvacc[hp][0:HD, :], ones64[HD:HD + 1, :],
                        rcpb[HD:HD + 1, hp, :])
                    if hp == 0:
                        nc.vector.tensor_mul(
                            oT[0:HD, pair, ccols], onum[0:HD, hp, :],
                            pvacc[hp][0:HD, :])
                    else:
                        tmp = npool.tile([HD, CHW], BF16, tag="otmp",
                                         name="otmp")
                        nc.vector.tensor_mul(
                            tmp, onum[0:HD, hp, :], pvacc[hp][0:HD, :])
                        nc.tensor.matmul(
                            pvacc[hp][HD:HD + HD, :], ident, tmp)
                        nc.vector.tensor_copy(
                            out=oT[HD:P, pair, ccols],
                            in_=pvacc[hp][HD:HD + HD, :])

            # out-projection for chunk c is released ~2 chunk-pairs after
            # its normalize: the early windows are PE-bound (exp waits on
            # the PE), while chunk 3's long exp phases leave the PE idle
            # enough to re-throttle its clock — shifting the out-projection
            # work there shortens the early windows and keeps the PE warm
            # late. Only outproj(3) remains for the tail.
            release = {(2, 1): 0, (3, 0): 1, (3, 1): 2}
            cells3 = None
            for c in range(NCH):
                for pair in (0, 1):
                    if (c, pair) in release:
                        _outproj_chunk(release[(c, pair)])
                    if (c, pair) == (3, 1):
                        cells3 = _outproj_gc0(3)
                    f.drain_until(("q", pair, c))
                    pvacc = _attn_chunk(c, pair)
                    _normalize(c, pair, pvacc)
            _outproj_gc1(3, cells3)
            f.drain()


def _fix_instruction_waits(nc):
    """Some lowered ISA structs (fp32r matmul LDW, DMA pseudo) carry at most
    one sync wait. Normalize: hoist excess waits onto NoOps inserted
    immediately before the instruction in the scheduled stream (same engine,
    so program order preserves the wait semantics)."""
    fixed = 0
    for blk in nc.m.functions[0].blocks:
        insts = blk.instructions
        idx = 0
        while idx < len(insts):
            inst = insts[idx]
            si = getattr(inst, "sync_info", None)
            if si is not None and len(si.on_wait) > 1:
                waits = list(si.on_wait)
                for j, wt in enumerate(waits[:-1]):
                    nop = mybir.InstNoOp(
                        name=f"I-wfix{fixed}-{j}-{inst.name}",
                        engine=inst.engine,
                        sync_info=mybir.SyncInfo(on_wait=[wt], on_update=[]))
                    insts.insert(idx, nop)
                    idx += 1
                inst.sync_info = mybir.SyncInfo(
                    on_wait=[waits[-1]], on_update=list(si.on_update))
                fixed += 1
            idx += 1
    return fixed


def _build():
    global _NC_CACHE
    if _NC_CACHE is None:
        nc = bass.Bass()
        with tile.TileContext(nc) as tc:
            _emit(tc)
        _fix_instruction_waits(nc)
        _NC_CACHE = nc
    return _NC_CACHE


def kernel(x, Wq, Wkv, Wo):
    global LAST_RESULTS
    x = np.asarray(x, dtype=np.float32)
    Wq = np.asarray(Wq, dtype=np.float32)
    Wkv = np.asarray(Wkv, dtype=np.float32)
    Wo = np.asarray(Wo, dtype=np.float32)

    nc = _build()
    bf = ml_dtypes.bfloat16
    in_maps = []
    for c in range(8):
        b, g = divmod(c, 4)
        cs = slice(GC * g, GC * (g + 1))
        in_maps.append({
            "xT": np.ascontiguousarray(x[b].T).astype(bf),
            "wq": np.ascontiguousarray(Wq[:, cs]).astype(bf),
            "wk": np.ascontiguousarray(Wkv[:, 0:D][:, cs]).astype(bf),
            "wv": np.ascontiguousarray(Wkv[:, D:2 * D][:, cs]).astype(bf),
            "wo": np.ascontiguousarray(Wo[cs, :]).astype(bf),
        })

    trace = os.environ.get("ATTN_KERNEL_TRACE", "0") == "1"
    res = run_bass_kernel_spmd(nc, in_maps, list(range(8)), trace=trace)
    LAST_RESULTS = res

    out = np.zeros((B, S, D), dtype=np.float32)
    for c in range(8):
        b = c // 4
        out[b] += res.results[c]["y"].astype(np.float32)
    return out


if __name__ == "__main__":
    rng = np.random.default_rng(0)
    s = 1.0 / np.sqrt(D)
    inputs = {
        "x": rng.standard_normal((B, S, D), dtype=np.float32),
        "Wq": rng.standard_normal((D, D), dtype=np.float32) * s,
        "Wkv": rng.standard_normal((D, 2 * D), dtype=np.float32) * s,
        "Wo": rng.standard_normal((D, D), dtype=np.float32) * s,
    }
    out = kernel(**inputs)
    print("out", out.shape, out.dtype, float(np.abs(out).mean()))


# revision 34
# speedup vs baseline: 1.0340x; 1.0340x over previous
"""Multi-head causal attention (B=2, S=2048, D=1024, H=16) on 8 TRN2 NeuronCores.

Sharding: core c handles batch b = c//4 and head-group g = c%4 (4 heads, 256 dims).
Each core computes Q/K/V projections for its head group from x[b], runs causal
attention per head, and applies its 256 rows of Wo, producing a partial [S, D]
output (bf16). The host sums the 4 head-group partials per batch in fp32.

Device schedule (per core); matmul operands bf16, accumulation fp32 in PSUM.
One sweep over the four 512-wide i-chunks, the two head-pairs interleaved per
chunk so the ScalarE exp stream and the TensorE matmul stream both stay dense:

  for c in 0..3:  for pair in 0,1:
    scores S^T[j,i] per 128-row j-block, both heads issued back-to-back on
    disjoint PE row groups (K=64 pairing); 1-group-ahead software pipelining
    (scores for block g+1 are emitted before PV of block g so the PE never
    heads-of-line-waits on the exp)
    P~^T = exp(scale*S^T) (ScalarE, 2 strips per instruction)
    diagonal blocks causal-masked in-place on GpSimd (affine_select -> 0)
    O'^T[65,i] += V'_j^T @ P~^T_j  (PSUM accumulate; row 64 = softmax denom)
    normalize: copy nums+den to SBUF, reciprocal_approx_fast on the denom row,
    DRAM round-trip broadcasts the recip across the 64 head-dim partitions,
    oT = num * recip (head 1 of the pair lands via a partition-shift DMA)
  after both pairs of chunk c: y rows = O @ Wo (PSUM accumulate over pairs),
  copied out as bf16 and DMA'd per 128-row block.

Q/K/V and output projections are emitted as ordered "filler" work drained
between attention blocks to fill the PE's slack under the exp stream.
A warmup burst of dummy matmuls plus an early dummy activation run during the
input DMA phase so the PE's HAM clock-gate is released (2.4 GHz) and the exp
table is resident before real work arrives.
"""

import os
from collections import deque

import ml_dtypes
import numpy as np

import concourse.bass as bass
import concourse.mybir as mybir
import concourse.tile as tile
from concourse.bass_utils import run_bass_kernel_spmd

F32 = mybir.dt.float32
BF16 = mybir.dt.bfloat16

B, S, D, H = 2, 2048, 1024, 16
HD = 64                     # head dim
GH = 4                      # heads per core
GC = GH * HD                # 256 projection cols per core
P = 128
KD = D // P                 # 8 contraction chunks for projections
NSB = S // P                # 16 seq blocks
CHW = 512                   # i-chunk width
NCH = S // CHW              # 4 i-chunks
SCALE = HD ** -0.5

_NC_CACHE = None
LAST_RESULTS = None         # BassKernelResults of the most recent run (for test.py)


class _Fillers:
    """Queue of small emission closures (1-2 TensorE ops each) drained
    between attention strip groups to keep the PE busy while ScalarE
    works through the exp stream. Markers let the consumer force-drain
    the prefix a dependent phase needs."""

    def __init__(self):
        self.q = deque()       # static projection work, with markers
        self.hq = deque()      # dynamic work (out-projection), served first
        self.seen = set()      # markers already popped (by step or drains)
        self.keepwarm = None   # fallback emitter for empty queues
        self.kw_budget = 0

    def add(self, fn):
        self.q.append(fn)

    def add_hq(self, fn):
        self.hq.append(fn)

    def add_marker(self, key):
        self.q.append(key)

    def _emit_q_one(self):
        item = self.q.popleft()
        if callable(item):
            item()
            return None
        self.seen.add(item)
        return item

    def step(self, n):
        done = 0
        use_hq = True
        while done < n:
            if not (self.q or self.hq):
                # keep the PE's HAM clock-gate from re-throttling during
                # ScalarE-bound stretches: emit capped junk matmuls
                if self.keepwarm is not None and self.kw_budget > 0:
                    self.kw_budget -= 1
                    self.keepwarm()
                    done += 1
                    continue
                break
            if self.hq and (use_hq or not self.q):
                self.hq.popleft()()
                done += 1
            elif self.q:
                if self._emit_q_one() is None:
                    done += 1
            use_hq = not use_hq

    def drain_until(self, key):
        if key in self.seen:
            return
        while self.q:
            if self._emit_q_one() == key:
                return

    def drain(self):
        while self.q or self.hq:
            if self.hq:
                self.hq.popleft()()
            if self.q:
                self._emit_q_one()


def _emit(tc):
    nc = tc.nc
    xT = nc.dram_tensor("xT", [D, S], BF16, kind="ExternalInput")
    wq = nc.dram_tensor("wq", [D, GC], BF16, kind="ExternalInput")
    wk = nc.dram_tensor("wk", [D, GC], BF16, kind="ExternalInput")
    wv = nc.dram_tensor("wv", [D, GC], BF16, kind="ExternalInput")
    wo = nc.dram_tensor("wo", [GC, D], BF16, kind="ExternalInput")
    y = nc.dram_tensor("y", [S, D], BF16, kind="ExternalOutput")

    xT_t = xT[:].rearrange("(o p) s -> p o s", p=P)      # [128, 8, S]
    wq_t = wq[:].rearrange("(o p) c -> p o c", p=P)      # [128, 8, 256]
    wk_t = wk[:].rearrange("(o p) c -> p o c", p=P)
    wv_t = wv[:].rearrange("(o p) c -> p o c", p=P)
    wo_t = wo[:].rearrange("(o p) n -> p o n", p=P)      # [128, 2, 1024]

    from contextlib import ExitStack

    with ExitStack() as top:
        persist = top.enter_context(tc.tile_pool(name="persist", bufs=1))

        wdum = persist.tile([P, P], BF16)
        nc.vector.memset(wdum, 0.0)
        ones_bf = persist.tile([P, 1], BF16)
        nc.vector.memset(ones_bf, 1.0)
        ones64 = persist.tile([P, HD], BF16)      # K=1 broadcast lhsT rows
        nc.vector.memset(ones64, 1.0)
        ident = persist.tile([HD, HD], BF16)      # partition-shift identity
        nc.gpsimd.memset(ident, 1.0)
        nc.gpsimd.affine_select(
            out=ident, in_=ident, compare_op=mybir.AluOpType.is_equal,
            fill=0.0, base=0, pattern=[[-1, HD]], channel_multiplier=1)

        wq_sb = persist.tile([P, KD, GC], BF16)
        wk_sb = persist.tile([P, KD, GC], BF16)
        wv_sb = persist.tile([P, KD, GC], BF16)
        wo_sb = persist.tile([P, 2, D], BF16)
        xfull = persist.tile([P, KD, S], BF16)

        # input DMAs split across the sync/scalar HWDGE queues and the
        # gpsimd SWDGE, in strict first-needed order: Q/K weights and x
        # chunk 0 (so the first projections can start ~12us in), then wv,
        # then the later x chunks. Scalar only carries early issues so
        # the exp stream is undisturbed once attention starts.
        nc.sync.dma_start(out=wq_sb, in_=wq_t)
        nc.scalar.dma_start(out=wk_sb, in_=wk_t)
        for k in range(KD):
            eng = nc.sync if k % 2 == 0 else nc.scalar
            eng.dma_start(out=xfull[:, k, 0:CHW], in_=xT_t[:, k, 0:CHW])
        nc.sync.dma_start(out=wv_sb, in_=wv_t)
        for ch in range(1, NCH):
            nc.sync.dma_start(
                out=xfull[:, 0:4, ch * CHW:(ch + 1) * CHW],
                in_=xT_t[:, 0:4, ch * CHW:(ch + 1) * CHW])
            nc.scalar.dma_start(
                out=xfull[:, 4:8, ch * CHW:(ch + 1) * CHW],
                in_=xT_t[:, 4:8, ch * CHW:(ch + 1) * CHW])
        nc.scalar.dma_start(out=wo_sb, in_=wo_t)

        qT = persist.tile([P, 2, S], BF16)               # [pair-cols, pair, seq]
        kT = persist.tile([P, 2, S], BF16)
        v_sb = persist.tile([P, NSB, GH, HD + 1], BF16)  # ones col appended
        oT = persist.tile([P, 2, S], BF16)
        nc.vector.tensor_copy(
            out=v_sb[:, :, :, HD:HD + 1],
            in_=ones_bf[:, 0:1].to_broadcast((P, NSB, GH, 1)))

        with ExitStack() as ph_b:
            ps_sc = ph_b.enter_context(
                tc.tile_pool(name="ps_sc", bufs=2, space="PSUM"))
            ps_pv = ph_b.enter_context(
                tc.tile_pool(name="ps_pv", bufs=1, space="PSUM"))
            ps_fill = ph_b.enter_context(
                tc.tile_pool(name="ps_fill", bufs=2, space="PSUM"))
            ppool = ph_b.enter_context(tc.tile_pool(name="pstrip", bufs=3))
            npool = ph_b.enter_context(tc.tile_pool(name="norm", bufs=6))
            opool = ph_b.enter_context(tc.tile_pool(name="onum", bufs=4))
            ypool = ph_b.enter_context(tc.tile_pool(name="ystage", bufs=6))
            ypool32 = ph_b.enter_context(tc.tile_pool(name="ystage32", bufs=4))

            # --- warmup: release the PE clock gate and preload the exp
            # table while the input DMAs are in flight ---
            warm_act = persist.tile([P, 8], F32)
            nc.scalar.activation(
                warm_act, wdum[:, 0:8], mybir.ActivationFunctionType.Exp)
            wt = ps_fill.tile([P, P], F32, tag="fill", name="warm")
            NWARM = 90
            for i in range(NWARM):
                nc.tensor.matmul(wt, wdum, wdum,
                                 start=(i == 0), stop=(i == NWARM - 1))

            f = _Fillers()

            def _keepwarm_item():
                kw = ps_fill.tile([P, P], F32, tag="fill", name="kw")
                nc.tensor.matmul(kw, wdum, wdum)

            f.keepwarm = _keepwarm_item
            f.kw_budget = 0

            def _qk_chunk(which, pair_, ch):
                # which: 0=Q, 1=K; emits 8 accumulating matmuls + copy-out
                cell = {}
                w_sb = wq_sb if which == 0 else wk_sb
                dst = qT if which == 0 else kT

                def alloc_mm(k, cell=cell, ch=ch, w_sb=w_sb, pair_=pair_):
                    if k == 0:
                        cell["p"] = ps_fill.tile(
                            [P, CHW], F32, tag="fill", name="fillqk")
                    nc.tensor.matmul(
                        cell["p"], w_sb[:, k, pair_ * P:(pair_ + 1) * P],
                        xfull[:, k, ch * CHW:(ch + 1) * CHW],
                        start=(k == 0), stop=(k == KD - 1))

                def copy(cell=cell, ch=ch, dst=dst, pair_=pair_):
                    nc.vector.tensor_copy(
                        out=dst[:, pair_, ch * CHW:(ch + 1) * CHW],
                        in_=cell["p"])

                for k in range(KD):
                    f.add(lambda k=k: alloc_mm(k))
                f.add(copy)

            def _v_block(sb):
                cell = {}

                def alloc_mm(k, cell=cell, sb=sb):
                    if k == 0:
                        cell["pv"] = ps_fill.tile(
                            [P, CHW], F32, tag="fill", name="fillpv")
                    nc.tensor.matmul(
                        cell["pv"][:, 0:GC],
                        xfull[:, k, sb * P:(sb + 1) * P], wv_sb[:, k, :],
                        start=(k == 0), stop=(k == KD - 1))

                def copy(cell=cell, sb=sb):
                    nc.vector.tensor_copy(
                        out=v_sb[:, sb, :, 0:HD],
                        in_=cell["pv"][:, 0:GC].rearrange(
                            "p (h d) -> p h d", h=GH))

                for k in range(KD):
                    f.add(lambda k=k: alloc_mm(k))
                f.add(copy)

            def _outproj_gc0(c):
                # first half of the last chunk's output projection: the
                # pair-0 contraction runs during (3,1) attention into f32
                # staging, so the tail only runs the pair-1 matmuls + adds
                cells = {}
                for s4 in range(CHW // P):
                    sb = c * (CHW // P) + s4
                    cell = {}
                    cells[sb] = cell

                    def ph1(cell=cell, sb=sb):
                        cell["y32"] = ypool32.tile(
                            [P, D], F32, tag="y32", name="y32")

                    f.add_hq(ph1)
                    for nch in range(2):
                        def mm0(cell=cell, sb=sb, nch=nch):
                            cell["py"] = ps_fill.tile(
                                [P, CHW], F32, tag="fill", name="fillpy")
                            nc.tensor.matmul(
                                cell["py"], oT[:, 0, sb * P:(sb + 1) * P],
                                wo_sb[:, 0, nch * CHW:(nch + 1) * CHW])

                        def cp0(cell=cell, nch=nch):
                            nc.vector.tensor_copy(
                                out=cell["y32"][:, nch * CHW:(nch + 1) * CHW],
                                in_=cell["py"])

                        f.add_hq(mm0)
                        f.add_hq(cp0)
                return cells

            def _outproj_gc1(c, cells):
                for s4 in range(CHW // P):
                    sb = c * (CHW // P) + s4
                    cell = cells[sb]

                    def alloc(cell=cell):
                        cell["ysb"] = ypool.tile(
                            [P, D], BF16, tag="ysb", name="ysb")

                    f.add_hq(alloc)
                    for nch in range(2):
                        def mm1(cell=cell, sb=sb, nch=nch):
                            cell["py2"] = ps_fill.tile(
                                [P, CHW], F32, tag="fill", name="fillpy2")
                            nc.tensor.matmul(
                                cell["py2"], oT[:, 1, sb * P:(sb + 1) * P],
                                wo_sb[:, 1, nch * CHW:(nch + 1) * CHW])

                        def addcp(cell=cell, nch=nch):
                            nc.vector.tensor_add(
                                cell["ysb"][:, nch * CHW:(nch + 1) * CHW],
                                cell["y32"][:, nch * CHW:(nch + 1) * CHW],
                                cell["py2"])

                        f.add_hq(mm1)
                        f.add_hq(addcp)

                    def out_dma(cell=cell, sb=sb):
                        eng = nc.sync if sb % 2 == 0 else nc.scalar
                        eng.dma_start(
                            out=y[sb * P:(sb + 1) * P, :], in_=cell["ysb"])

                    f.add_hq(out_dma)

            def _outproj_chunk(c):
                for s4 in range(CHW // P):
                    sb = c * (CHW // P) + s4
                    cell = {}

                    def alloc(cell=cell):
                        cell["ysb"] = ypool.tile(
                            [P, D], BF16, tag="ysb", name="ysb")

                    f.add_hq(alloc)
                    for nch in range(2):
                        def mm(gc, cell=cell, sb=sb, nch=nch):
                            if gc == 0:
                                cell["py"] = ps_fill.tile(
                                    [P, CHW], F32, tag="fill", name="fillpy")
                            nc.tensor.matmul(
                                cell["py"], oT[:, gc, sb * P:(sb + 1) * P],
                                wo_sb[:, gc, nch * CHW:(nch + 1) * CHW],
                                start=(gc == 0), stop=(gc == 1))

                        def cp(cell=cell, nch=nch):
                            nc.vector.tensor_copy(
                                out=cell["ysb"][:, nch * CHW:(nch + 1) * CHW],
                                in_=cell["py"])

                        f.add_hq(lambda mm=mm: mm(0))
                        f.add_hq(lambda mm=mm: mm(1))
                        f.add_hq(cp)

                    def out_dma(cell=cell, sb=sb):
                        nc.sync.dma_start(
                            out=y[sb * P:(sb + 1) * P, :], in_=cell["ysb"])

                    f.add_hq(out_dma)

            # projection work, in first-needed order. Markers are placed so
            # a chunk only force-drains Q at its start; K and the V blocks
            # are only forced right before the diagonal groups that first
            # need them, letting them spread across the chunk's groups.
            for ch in range(NCH):
                _qk_chunk(0, 0, ch)
                f.add_marker(("q", 0, ch))
                _qk_chunk(1, 0, ch)
                f.add_marker(("k", 0, ch))
                for s4 in range(CHW // P):
                    _v_block(ch * (CHW // P) + s4)
                f.add_marker(("v", ch))
                _qk_chunk(0, 1, ch)
                f.add_marker(("q", 1, ch))
                _qk_chunk(1, 1, ch)
                f.add_marker(("k", 1, ch))

            def _attn_chunk(c, pair):
                njb = 4 * c + 4
                pvacc = {
                    0: ps_pv.tile([P, CHW], F32, tag="pv0", name="pv0"),
                    1: ps_pv.tile([P, CHW], F32, tag="pv1", name="pv1"),
                }
                scs = {}

                def emit_scores(jb):
                    if jb == 4 * c:     # diagonal: K(pair,c) now needed
                        f.drain_until(("k", pair, c))
                    tl = max(0, jb - 4 * c) * P
                    sc = ps_sc.tile([P, 2, CHW], F32, tag="sc", name="sc")
                    for hp in (0, 1):
                        bp = hp * HD
                        nc.tensor.matmul(
                            sc[:, hp, tl:],
                            kT[bp:bp + HD, pair, jb * P:(jb + 1) * P],
                            qT[bp:bp + HD, pair, c * CHW + tl:(c + 1) * CHW])
                    scs[jb] = (sc, tl)

                emit_scores(0)
                for jb in range(njb):
                    sc, tl = scs.pop(jb)
                    pt = ppool.tile([P, 2, CHW], BF16, tag="pt", name="pt")
                    nc.scalar.activation(
                        pt[:, :, tl:], sc[:, :, tl:],
                        mybir.ActivationFunctionType.Exp, scale=SCALE)
                    if jb + 1 < njb:
                        emit_scores(jb + 1)
                    if jb >= 4 * c:           # diagonal block: causal mask
                        nc.gpsimd.affine_select(
                            out=pt[:, :, tl:tl + P], in_=pt[:, :, tl:tl + P],
                            compare_op=mybir.AluOpType.is_ge, fill=0.0,
                            base=0, pattern=[[0, 2], [1, P]],
                            channel_multiplier=-1)
                    if pair == 0 and jb == 4 * c:   # diag PV needs V(c)
                        f.drain_until(("v", c))
                    f.step(5)
                    for hp in (0, 1):
                        h = pair * 2 + hp
                        nc.tensor.matmul(
                            pvacc[hp][0:HD + 1, tl:], v_sb[:, jb, h, :],
                            pt[:, hp, tl:],
                            start=(jb == 0), stop=(jb == njb - 1))
                return pvacc

            def _normalize(c, pair, pvacc):
                # DMA-free normalize (every chunk): copy nums+den to SBUF,
                # reciprocal of the two denominator rows as exp(-ln(den))
                # on ScalarE (one batched instruction each; Ln and Exp
                # share the natural_log_exp table set), partition-broadcast
                # of the recip via a K=1 TensorE matmul into the freed
                # pvacc bank, and the head-1 partition shift via an
                # identity matmul. The ~10us-per-hop DMA latency of the
                # runtime never enters the oT dependency chain.
                ccols = slice(c * CHW, (c + 1) * CHW)
                onum = opool.tile([HD + 1, 2, CHW], F32, tag="on", name="onum")
                for hp in (0, 1):
                    nc.scalar.activation(
                        onum[HD:HD + 1, hp, :], pvacc[hp][HD:HD + 1, :],
                        mybir.ActivationFunctionType.Ln)
                    nc.vector.tensor_copy(
                        out=onum[0:HD, hp, :], in_=pvacc[hp][0:HD, :])
                den = onum[HD:HD + 1, :, :]
                rcpb = opool.tile([HD + 1, 2, CHW], BF16, tag="rcb",
                                  name="rcpb")
                nc.scalar.activation(
                    rcpb[HD:HD + 1, :, :], den,
                    mybir.ActivationFunctionType.Exp, scale=-1.0)
                for hp in (0, 1):
                    # broadcast recip across the 64 head-dim rows into the
                    # freed pvacc bank (start=True only clears has_written
                    # bits; no live data remains there); bf16 operands keep
                    # the matmul single-pass
                    nc.tensor.matmul(
                        pvacc[hp][0:HD, :], ones64[HD:HD + 1, :],
                        rcpb[HD:HD + 1, hp, :])
                    if hp == 0:
                        nc.vector.tensor_mul(
                            oT[0:HD, pair, ccols], onum[0:HD, hp, :],
                            pvacc[hp][0:HD, :])
                    else:
                        tmp = npool.tile([HD, CHW], BF16, tag="otmp",
                                         name="otmp")
                        nc.vector.tensor_mul(
                            tmp, onum[0:HD, hp, :], pvacc[hp][0:HD, :])
                        nc.tensor.matmul(
                            pvacc[hp][HD:HD + HD, :], ident, tmp)
                        nc.vector.tensor_copy(
                            out=oT[HD:P, pair, ccols],
                            in_=pvacc[hp][HD:HD + HD, :])

            # out-projection for chunk c is released ~2 chunk-pairs after
            # its normalize: the early windows are PE-bound (exp waits on
            # the PE), while chunk 3's long exp phases leave the PE idle
            # enough to re-throttle its clock — shifting the out-projection
            # work there shortens the early windows and keeps the PE warm
            # late. Only outproj(3) remains for the tail.
            release = {(2, 1): 0, (3, 0): 1, (3, 1): 2}
            cells3 = None
            for c in range(NCH):
                for pair in (0, 1):
                    if (c, pair) in release:
                        _outproj_chunk(release[(c, pair)])
                    if (c, pair) == (3, 1):
                        cells3 = _outproj_gc0(3)
                    f.drain_until(("q", pair, c))
                    pvacc = _attn_chunk(c, pair)
                    _normalize(c, pair, pvacc)
            _outproj_gc1(3, cells3)
            f.drain()


def _fix_instruction_waits(nc):
    """Some lowered ISA structs (fp32r matmul LDW, DMA pseudo) carry at most
    one sync wait. Normalize: hoist excess waits onto NoOps inserted
    immediately before the instruction in the scheduled stream (same engine,
    so program order preserves the wait semantics)."""
    fixed = 0
    for blk in nc.m.functions[0].blocks:
        insts = blk.instructions
        idx = 0
        while idx < len(insts):
            inst = insts[idx]
            si = getattr(inst, "sync_info", None)
            if si is not None and len(si.on_wait) > 1:
                waits = list(si.on_wait)
                for j, wt in enumerate(waits[:-1]):
                    nop = mybir.InstNoOp(
                        name=f"I-wfix{fixed}-{j}-{inst.name}",
                        engine=inst.engine,
                        sync_info=mybir.SyncInfo(on_wait=[wt], on_update=[]))
                    insts.insert(idx, nop)
                    idx += 1
                inst.sync_info = mybir.SyncInfo(
                    on_wait=[waits[-1]], on_update=list(si.on_update))
                fixed += 1
            idx += 1
    return fixed


def _build():
    global _NC_CACHE
    if _NC_CACHE is None:
        nc = bass.Bass()
        with tile.TileContext(nc) as tc:
            _emit(tc)
        _fix_instruction_waits(nc)
        _NC_CACHE = nc
    return _NC_CACHE


def kernel(x, Wq, Wkv, Wo):
    global LAST_RESULTS
    x = np.asarray(x, dtype=np.float32)
    Wq = np.asarray(Wq, dtype=np.float32)
    Wkv = np.asarray(Wkv, dtype=np.float32)
    Wo = np.asarray(Wo, dtype=np.float32)

    nc = _build()
    bf = ml_dtypes.bfloat16
    in_maps = []
    for c in range(8):
        b, g = divmod(c, 4)
        cs = slice(GC * g, GC * (g + 1))
        in_maps.append({
            "xT": np.ascontiguousarray(x[b].T).astype(bf),
            "wq": np.ascontiguousarray(Wq[:, cs]).astype(bf),
            "wk": np.ascontiguousarray(Wkv[:, 0:D][:, cs]).astype(bf),
            "wv": np.ascontiguousarray(Wkv[:, D:2 * D][:, cs]).astype(bf),
            "wo": np.ascontiguousarray(Wo[cs, :]).astype(bf),
        })

    trace = os.environ.get("ATTN_KERNEL_TRACE", "0") == "1"
    res = run_bass_kernel_spmd(nc, in_maps, list(range(8)), trace=trace)
    LAST_RESULTS = res

    out = np.zeros((B, S, D), dtype=np.float32)
    for c in range(8):
        b = c // 4
        out[b] += res.results[c]["y"].astype(np.float32)
    return out


if __name__ == "__main__":
    rng = np.random.default_rng(0)
    s = 1.0 / np.sqrt(D)
    inputs = {
        "x": rng.standard_normal((B, S, D), dtype=np.float32),
        "Wq": rng.standard_normal((D, D), dtype=np.float32) * s,
        "Wkv": rng.standard_normal((D, 2 * D), dtype=np.float32) * s,
        "Wo": rng.standard_normal((D, D), dtype=np.float32) * s,
    }
    out = kernel(**inputs)
    print("out", out.shape, out.dtype, float(np.abs(out).mean()))
